# revision 5
# baseline (speedup 1.0000x reference)
"""GraphSAGE 2-layer GNN kernel for 8 TRN2 NeuronCores.

Strategy: destination-shard the 100K nodes across 8 cores (12.5K each).
Layer-1 messages x[src] are materialized host-side into a dense bf16 table in
(degree-class, segment)-packed layout; the device streams it densely.
Segment sums run as a pairwise tensor_add tree on the DVE (contiguous bf16,
2x mode) instead of strided tensor_reduce.  h is AllGathered in bf16; layer-2
messages are fetched from the shared h table with chunk-batched indirect DMAs
(one instruction per chunk, [128 x MC] offset AP), then reduced/transformed
identically.
"""
import sys
sys.path.insert(0, '/opt/trn_rl_repo')
import numpy as np
import ml_dtypes

import concourse.bass as bass
import concourse.tile as tile
from concourse import bacc, mybir
from concourse.bass_utils import run_bass_kernel_spmd
from concourse.masks import make_identity

N_CORES = 8
N_NODES = 100000
D = 128
SHARD = N_NODES // N_CORES  # 12500
# every class halves repeatedly to 3 or 1 (handled by the TT tree)
CLASSES = [2, 4, 6, 8, 12, 16, 24, 32, 48, 64, 96, 128]
MC = 160   # msg cols per chunk, both layers (bf16)
GK = 32    # cols per indirect_dma_start in the layer-2 gather (divides MC)

BF16 = ml_dtypes.bfloat16


def _class_of(deg):
    for L in CLASSES:
        if deg <= L:
            return L
    raise AssertionError(f"degree {deg} exceeds max class")


def _pack_cores(per_core_dsts_deg, mc):
    """Pack every core's dst segments into ONE common (class, slot, partition)
    layout so the SPMD program is identical across cores."""
    per_core_members = []
    for dd in per_core_dsts_deg:
        m = {L: [] for L in CLASSES}
        for node, deg in dd:
            m[_class_of(deg)].append(node)
        per_core_members.append(m)

    slots_per_class = {}
    for L in CLASSES:
        n = max(len(m[L]) for m in per_core_members)
        slots_per_class[L] = (n + 127) // 128

    blocks_per_core = []
    for m in per_core_members:
        blocks = []       # (L, [node or -1]*128) per (class, slot)
        for L in CLASSES:
            n_slots = slots_per_class[L]
            nodes = m[L] + [-1] * (n_slots * 128 - len(m[L]))
            for s in range(n_slots):
                blocks.append((L, nodes[s * 128:(s + 1) * 128]))
        blocks_per_core.append(blocks)

    class_of_block = []
    for L in CLASSES:
        class_of_block += [L] * slots_per_class[L]

    plan = []          # (msg_col, L, block_index)
    cur = 0
    for b, L in enumerate(class_of_block):
        if cur % mc + L > mc:
            cur += mc - (cur % mc)    # pad to chunk boundary
        plan.append((cur, L, b))
        cur += L
    m_total = cur + ((-cur) % mc)
    return blocks_per_core, plan, m_total


def _chunk_runs(plan, mc):
    """Group plan entries into per-chunk runs of consecutive same-class
    blocks: {chunk: [(local_col, L, b0, nblk)]}."""
    runs = {}
    for (col, L, b) in plan:
        k = col // mc
        lc = col - k * mc
        lst = runs.setdefault(k, [])
        if lst and lst[-1][1] == L and lst[-1][0] + lst[-1][3] * L == lc \
                and lst[-1][2] + lst[-1][3] == b:
            lst[-1][3] += 1
        else:
            lst.append([lc, L, b, 1])
    return runs


def _build_tables(x, edge_index):
    """All host-side preprocessing. Returns per-core input dicts + metadata."""
    src = np.asarray(edge_index[0], dtype=np.int64)
    dst = np.asarray(edge_index[1], dtype=np.int64)
    deg = np.bincount(dst, minlength=N_NODES).astype(np.int64)

    order = np.argsort(dst, kind='stable')
    src_sorted = src[order]
    dst_starts = np.zeros(N_NODES + 1, dtype=np.int64)
    np.cumsum(deg, out=dst_starts[1:])

    per_core_dd = []
    for c in range(N_CORES):
        lo, hi = c * SHARD, (c + 1) * SHARD
        per_core_dd.append([(int(n), int(deg[n])) for n in range(lo, hi)])
    blocks_per_core, plan, m_total = _pack_cores(per_core_dd, MC)
    cores = [{"blocks": b} for b in blocks_per_core]

    nseg = len(blocks_per_core[0])
    shard_rows = nseg * 128 + 128          # + one zero-row block

    x_bf = x.astype(BF16)
    invdeg = 1.0 / np.maximum(deg.astype(np.float32), 1.0)

    # global node -> h_full row id
    node_row = np.full(N_NODES, -1, dtype=np.int64)
    for c, ci in enumerate(cores):
        for b, (_L, nodes) in enumerate(ci["blocks"]):
            nn = np.array(nodes, dtype=np.int64)
            v = nn >= 0
            node_row[nn[v]] = c * shard_rows + b * 128 + np.nonzero(v)[0]
    assert (node_row >= 0).all()

    for c, ci in enumerate(cores):
        blocks = ci["blocks"]
        zero_row = c * shard_rows + nseg * 128

        # slot (p, col) -> src node (layer-1 table) / h_full row (layer-2)
        slot_src = np.full((128, m_total), -1, dtype=np.int64)
        idx2 = np.full((128, m_total), zero_row, dtype=np.int32)

        node_of = np.full((128, nseg), -1, dtype=np.int64)
        for (col, Lc, b) in plan:
            L, nodes = blocks[b]
            assert L == Lc
            for p, n in enumerate(nodes):
                node_of[p, b] = n
                if n < 0:
                    continue
                s0, s1 = dst_starts[n], dst_starts[n + 1]
                srcs = src_sorted[s0:s1]
                k = len(srcs)
                slot_src[p, col:col + k] = srcs
                idx2[p, col:col + k] = node_row[srcs]

        t1 = np.zeros((128, m_total, D), dtype=BF16)
        valid = slot_src >= 0
        t1[valid] = x_bf[slot_src[valid]]

        inv_tile = np.ones((128, nseg), dtype=np.float32)
        xdT = np.zeros((128, nseg * 128), dtype=BF16)
        nv = node_of >= 0
        pp, bb = np.nonzero(nv)
        nodes_v = node_of[pp, bb]
        inv_tile[pp, bb] = invdeg[nodes_v]
        xdT[:, bb * 128 + pp] = x_bf[nodes_v].T

        ci["t1"] = t1
        ci["idx2"] = idx2
        ci["inv"] = inv_tile
        ci["xdT"] = xdT
        ci["node_of"] = node_of

    meta = {"nseg": nseg, "m": m_total, "plan": plan,
            "shard_rows": shard_rows, "cores": cores}
    return meta


def _build_program(meta):
    nseg, m = meta["nseg"], meta["m"]
    shard_rows = meta["shard_rows"]
    plan = meta["plan"]
    runs = _chunk_runs(plan, MC)
    n_chunks = m // MC
    blocks_of_chunk = {k: [(lc, L, b0, nb) for (lc, L, b0, nb) in runs.get(k, [])]
                       for k in range(n_chunks)}

    f32, bf16, i32 = mybir.dt.float32, mybir.dt.bfloat16, mybir.dt.int32
    nc = bacc.Bacc("TRN2", target_bir_lowering=False, debug=False,
                   num_devices=N_CORES)

    t1_d = nc.dram_tensor("t1", [128, m, D], bf16, kind="ExternalInput")
    idx2_d = nc.dram_tensor("idx2", [128, m], i32, kind="ExternalInput")
    inv_d = nc.dram_tensor("inv", [128, nseg], f32, kind="ExternalInput")
    xdT_d = nc.dram_tensor("xdT", [128, nseg * 128], bf16, kind="ExternalInput")
    w1l_d = nc.dram_tensor("w1lT", [128, 128], bf16, kind="ExternalInput")
    w1r_d = nc.dram_tensor("w1rT", [128, 128], bf16, kind="ExternalInput")
    w2l_d = nc.dram_tensor("w2lT", [128, 128], bf16, kind="ExternalInput")
    w2r_d = nc.dram_tensor("w2rT", [128, 128], bf16, kind="ExternalInput")
    b1_d = nc.dram_tensor("b1", [128, 1], f32, kind="ExternalInput")
    b2_d = nc.dram_tensor("b2", [128, 1], f32, kind="ExternalInput")
    outT_d = nc.dram_tensor("outT", [128, nseg * 128], f32, kind="ExternalOutput")

    h_shard = nc.dram_tensor("h_shard", [shard_rows, D], bf16)
    h_full = nc.dram_tensor("h_full", [N_CORES * shard_rows, D], bf16,
                            addr_space="Shared")

    with tile.TileContext(nc) as tc:
        with (
            tc.tile_pool(name="msg", bufs=2) as msg_pool,
            tc.tile_pool(name="scr", bufs=2) as scr_pool,
            tc.tile_pool(name="persist", bufs=1) as pp,
            tc.tile_pool(name="work", bufs=3) as wp,
            tc.tile_pool(name="psum", bufs=2, space="PSUM") as psp,
        ):
            agg = pp.tile([128, nseg, D], bf16, tag="agg")
            hT = pp.tile([128, nseg * 128], bf16, tag="hT")
            inv_t = pp.tile([128, nseg], f32, tag="inv")
            nc.sync.dma_start(out=inv_t[:], in_=inv_d.ap())
            idx2_t = pp.tile([128, m], i32, tag="idx2")
            nc.sync.dma_start(out=idx2_t[:], in_=idx2_d.ap())
            w1l = pp.tile([128, 128], bf16, tag="w1l")
            nc.sync.dma_start(out=w1l[:], in_=w1l_d.ap())
            w1r = pp.tile([128, 128], bf16, tag="w1r")
            nc.sync.dma_start(out=w1r[:], in_=w1r_d.ap())
            w2l = pp.tile([128, 128], bf16, tag="w2l")
            nc.sync.dma_start(out=w2l[:], in_=w2l_d.ap())
            w2r = pp.tile([128, 128], bf16, tag="w2r")
            nc.sync.dma_start(out=w2r[:], in_=w2r_d.ap())
            b1_t = pp.tile([128, 1], f32, tag="b1")
            nc.sync.dma_start(out=b1_t[:], in_=b1_d.ap())
            b2_t = pp.tile([128, 1], f32, tag="b2")
            nc.sync.dma_start(out=b2_t[:], in_=b2_d.ap())
            ident = pp.tile([128, 128], bf16, tag="ident")
            make_identity(nc, ident[:])

            def seg_view(t, col0, nblk, ln, lo, hi):
                """[128, nblk, (hi-lo)*128] view of segments of length ln
                starting at column col0: per-segment columns [lo, hi)."""
                v = t[:, col0:col0 + nblk * ln, :].rearrange(
                    "p (n l) f -> p n (l f)", n=nblk)
                return v[:, :, lo * 128:hi * 128]

            def tree_run(t, scr, lc, L, b0, nblk):
                """Pairwise-add reduce of one same-class run into agg."""
                if L == 2:
                    nc.vector.tensor_add(
                        agg[:, b0:b0 + nblk, :],
                        seg_view(t, lc, nblk, 2, 0, 1),
                        seg_view(t, lc, nblk, 2, 1, 2))
                    return
                cur, other = t, scr
                cur_off, ln = lc, L
                while True:
                    half = ln // 2
                    o_off = cur_off // 2 if other is scr else cur_off
                    if ln == 3:
                        tmp = seg_view(other, o_off, nblk, 1, 0, 1)
                        nc.vector.tensor_add(
                            tmp,
                            seg_view(cur, cur_off, nblk, 3, 0, 1),
                            seg_view(cur, cur_off, nblk, 3, 1, 2))
                        nc.vector.tensor_add(
                            agg[:, b0:b0 + nblk, :],
                            tmp,
                            seg_view(cur, cur_off, nblk, 3, 2, 3))
                        return
                    if ln == 2:
                        nc.vector.tensor_add(
                            agg[:, b0:b0 + nblk, :],
                            seg_view(cur, cur_off, nblk, 2, 0, 1),
                            seg_view(cur, cur_off, nblk, 2, 1, 2))
                        return
                    nc.vector.tensor_add(
                        seg_view(other, o_off, nblk, half, 0, half),
                        seg_view(cur, cur_off, nblk, ln, 0, half),
                        seg_view(cur, cur_off, nblk, ln, half, ln))
                    cur, other = other, cur
                    cur_off, ln = o_off, half

            def transform_block(b, wl, wr, bias_t, rhs, func, out_fn):
                mean_b = wp.tile([128, 128], bf16, tag="mean")
                nc.vector.tensor_scalar_mul(
                    mean_b[:], agg[:, b, :], inv_t[:, b:b + 1])
                mT_ps = psp.tile([128, 128], bf16, space="PSUM", tag="tp")
                nc.tensor.transpose(out=mT_ps[:], in_=mean_b[:],
                                    identity=ident[:])
                meanT = wp.tile([128, 128], bf16, tag="meanT")
                nc.scalar.copy(meanT[:], mT_ps[:])
                ps = psp.tile([128, 128], f32, space="PSUM", tag="mm")
                nc.tensor.matmul(out=ps[:], lhsT=wl[:], rhs=meanT[:],
                                 start=True, stop=False)
                nc.tensor.matmul(out=ps[:], lhsT=wr[:], rhs=rhs,
                                 start=False, stop=True)
                out_fn(b, ps, bias_t, func)

            def run_layer(get_chunk, wl, wr, bias_t, rhs_fn, func, out_fn):
                for k in range(n_chunks):
                    t = get_chunk(k)
                    scr = scr_pool.tile([128, MC // 2, D], bf16, tag="scr")
                    for (lc, L, b0, nblk) in blocks_of_chunk[k]:
                        tree_run(t, scr, lc, L, b0, nblk)
                    for (lc, L, b0, nblk) in blocks_of_chunk[k]:
                        for b in range(b0, b0 + nblk):
                            transform_block(b, wl, wr, bias_t, rhs_fn(b),
                                            func, out_fn)

            # ---------------- layer 1 ----------------
            def l1_chunk(k):
                t = msg_pool.tile([128, MC, D], bf16, tag="msg")
                nc.sync.dma_start(out=t[:], in_=t1_d.ap()[:, k * MC:(k + 1) * MC, :])
                return t

            def l1_rhs(b):
                xT_b = wp.tile([128, 128], bf16, tag="xTb")
                nc.sync.dma_start(out=xT_b[:], in_=xdT_d.ap()[:, b * 128:(b + 1) * 128])
                return xT_b[:]

            def l1_out(b, ps, bias_t, func):
                nc.scalar.activation(out=hT[:, b * 128:(b + 1) * 128], in_=ps[:],
                                     func=func, bias=bias_t[:], scale=1.0)
                hps = psp.tile([128, 128], bf16, space="PSUM", tag="tp2")
                nc.tensor.transpose(out=hps[:], in_=hT[:, b * 128:(b + 1) * 128],
                                    identity=ident[:])
                h_blk = wp.tile([128, 128], bf16, tag="hblk")
                nc.scalar.copy(h_blk[:], hps[:])
                nc.sync.dma_start(out=h_shard.ap()[b * 128:(b + 1) * 128, :],
                                  in_=h_blk[:])

            run_layer(l1_chunk, w1l, w1r, b1_t, l1_rhs,
                      mybir.ActivationFunctionType.Relu, l1_out)

            zt = wp.tile([128, 128], bf16, tag="zero")
            nc.vector.memset(zt[:], 0.0)
            nc.sync.dma_start(out=h_shard.ap()[nseg * 128:(nseg + 1) * 128, :],
                              in_=zt[:])

            nc.gpsimd.collective_compute(
                "AllGather", mybir.AluOpType.bypass,
                ins=[h_shard.ap().opt()], outs=[h_full.ap().opt()],
                replica_groups=[list(range(N_CORES))],
            )

            # ---------------- layer 2 ----------------
            def l2_chunk(k):
                t = msg_pool.tile([128, MC, D], bf16, tag="msg")
                for j0 in range(0, MC, GK):
                    for j in range(j0, j0 + GK):
                        nc.gpsimd.indirect_dma_start(
                            out=t[:, j, :], out_offset=None, in_=h_full.ap(),
                            in_offset=bass.IndirectOffsetOnAxis(
                                ap=idx2_t[:, k * MC + j:k * MC + j + 1], axis=0))
                return t

            def l2_rhs(b):
                return hT[:, b * 128:(b + 1) * 128]

            def l2_out(b, ps, bias_t, func):
                oT = wp.tile([128, 128], f32, tag="oT")
                nc.scalar.activation(out=oT[:], in_=ps[:], func=func,
                                     bias=bias_t[:], scale=1.0)
                nc.sync.dma_start(out=outT_d.ap()[:, b * 128:(b + 1) * 128],
                                  in_=oT[:])

            run_layer(l2_chunk, w2l, w2r, b2_t, l2_rhs,
                      mybir.ActivationFunctionType.Identity, l2_out)

    nc.compile()
    return nc


_CACHE = {}


def kernel(x, edge_index, W1_l, b1_l, W1_r, W2_l, b2_l, W2_r):
    x = np.asarray(x, dtype=np.float32)
    meta = _build_tables(x, np.asarray(edge_index))

    key = (meta["nseg"], meta["m"])
    if key not in _CACHE:
        _CACHE[key] = _build_program(meta)
    nc = _CACHE[key]

    in_maps = []
    for c in range(N_CORES):
        ci = meta["cores"][c]
        in_maps.append({
            "t1": ci["t1"], "idx2": ci["idx2"], "inv": ci["inv"],
            "xdT": ci["xdT"],
            "w1lT": np.asarray(W1_l, np.float32).T.astype(BF16).copy(),
            "w1rT": np.asarray(W1_r, np.float32).T.astype(BF16).copy(),
            "w2lT": np.asarray(W2_l, np.float32).T.astype(BF16).copy(),
            "w2rT": np.asarray(W2_r, np.float32).T.astype(BF16).copy(),
            "b1": np.asarray(b1_l, np.float32).reshape(128, 1).copy(),
            "b2": np.asarray(b2_l, np.float32).reshape(128, 1).copy(),
        })

    res = run_bass_kernel_spmd(nc, in_maps, core_ids=list(range(N_CORES)))

    out = np.zeros((N_NODES, D), dtype=np.float32)
    for c in range(N_CORES):
        outT = res.results[c]["outT"].reshape(128, meta["nseg"] * 128)
        node_of = meta["cores"][c]["node_of"]      # [128, nseg]
        pp_, bb = np.nonzero(node_of >= 0)
        nodes = node_of[pp_, bb]
        out[nodes] = outT[:, bb * 128 + pp_].T
    return out
